# revision 38
# baseline (speedup 1.0000x reference)
"""Two-layer GAT (GATConv(128->4x64, concat) + LayerNorm + ELU +
GATConv(256->2)) on 8 trn2 NeuronCores via Bass/Tile.

Distribution (graph/data parallel per the sharding hint): destination nodes
are partitioned across 8 cores; weights are replicated; each core computes the
dense node transform for all nodes (cheap in bf16) so no halo exchange is
needed for layer 0; the layer-1 node table is AllGathered once (compact 8B
rows) at the layer boundary and expanded locally to gather stride.

Per layer, per core (dst-partitioned):
- Edges (incl. self-loops) are grouped by destination 128-block, padded to a
  tile count shared across cores; within each (block, src-half) segment edges
  are sorted by (dst, src).  Source tables are split at gid 32768 (A/B halves)
  because gather indices are int16.
- dma_gather fetches one row per edge: layer 0 a 768B row [z bf16 x256 |
  as/ad fp32 bit-packed], layer 1 a 256B row [y0,y1 bf16 | as1 fp32packed].
- Per-edge dst-alpha comes from NO gather: tiles are dst-sorted, so the
  dst-selection one-hot S2[j,p] = (start[j]<=p<end[j]) is a staircase built
  by two DVE range-compares from tiny host tables; adE = S2^T @ adblock is a
  small PE matmul against the SBUF-resident per-block alpha-dst table.
- e = exp(leaky_relu(as + adE)); e is written into spare columns of the
  gathered rows so ONE matmul per tile produces aggregate + softmax denom:
  [agg | denom] += S^T @ [V*e | e] accumulated in PSUM per dst-block.
  (Softmax max-subtraction skipped: logits are O(5), exp safe in fp32.)
- Layer-0 flush: +b0, LayerNorm, ELU -> z, projected immediately to the
  layer-1 channels (y = z @ W1; valid by linearity) and attention scalars.
  y/as1 go to a compact [pcpad,4]-bf16 local table (AllGather payload 50KB);
  ad1 stays SBUF-resident.  After AllGather the compact table is expanded
  into a 256B-stride bf16 table for the layer-1 gather.

gids 0 and 32768 are reserved zero-pad nodes; pad gather slots read them (or
are negative-skipped + memset when cfg.negpad).
"""

import numpy as np

import concourse.bass as bass
import concourse.tile as tile
from concourse import bacc, mybir
from concourse.bass_utils import run_bass_kernel_spmd

F32 = mybir.dt.float32
BF16 = mybir.dt.bfloat16
I16 = mybir.dt.int16
ALU = mybir.AluOpType
ACTF = mybir.ActivationFunctionType


class CFG:
    def __init__(self, n_nodes=50000, split=32768, gb=2, gbufs=2, fbufs=2,
                 aggbufs=3, negpad=False, ch0=8, exch=32):
        self.n_nodes = n_nodes
        self.in_ch = 128
        self.hid = 64
        self.heads = 4
        self.out_ch = 2
        self.neg = 0.2
        self.eps = 1e-5
        self.n_cores = 8
        self.split = split
        self.gb = gb                 # dst-blocks per gather group
        self.gbufs = gbufs
        self.fbufs = fbufs
        self.aggbufs = aggbufs
        self.negpad = negpad         # pads use idx=-1 (skipped) + memset
        self.ch0 = ch0               # ph0 blocks per chunk
        self.exch = exch             # expand blocks per chunk
        self.val_w = 256             # value row width (elements)
        self.pc = n_nodes // self.n_cores
        self.bpc = (self.pc + 127) // 128
        self.pcpad = self.bpc * 128
        self.npad = self.n_cores * self.pcpad


cfg = CFG()


def configure(**kw):
    """Override module config (used by the test harness for mini runs)."""
    global cfg
    cfg = CFG(**kw)
    _cache.clear()


# ------------------------------------------------------------------ host ----

def _wrap_idx(idx):
    """[n] -> [128, n//16] int16: slot i at [i%16, i//16], replicated x8."""
    idx = np.asarray(idx, np.int16)
    n = idx.shape[0]
    assert n % 16 == 0
    w = idx.reshape(n // 16, 16).T
    return np.tile(w, (8, 1)).copy()


def _haddr(c, gid):
    """Value-table row address for a gid: p-major (transposed) layout so the
    ph0 writes and expand writes are contiguous per partition.
    addr(ci, blk, p) = p * nblk_tot + ci * bpc + blk."""
    gid = np.asarray(gid)
    ci = gid // c.pcpad
    l = gid % c.pcpad
    return (l % 128) * (c.n_cores * c.bpc) + ci * c.bpc + l // 128


def _gid_map(c):
    """[n_nodes] -> padded gid; the nodes whose table ADDRESS is 0 or
    c.split are reserved zero rows (out-of-half gather pads read them)."""
    nblk_tot = c.n_cores * c.bpc
    gids = np.zeros(c.n_nodes, np.int64)
    for ci in range(c.n_cores):
        base = ci * c.pcpad
        slots = np.arange(c.pcpad)
        forb = []
        for target in (0, c.split):
            p, K = target // nblk_tot, target % nblk_tot
            if K // c.bpc == ci:
                forb.append((K % c.bpc) * 128 + p)
        if forb:
            keep = np.ones(c.pcpad, bool)
            keep[forb] = False
            slots = slots[keep]
        gids[ci * c.pc:(ci + 1) * c.pc] = base + slots[:c.pc]
    return gids


def prep(x, edge_index, W0, a_src0, a_dst0, b0, ln_g, ln_b, W1,
         a_src1, a_dst1, b1):
    c = cfg
    N, C, H, HID = c.n_nodes, c.n_cores, c.heads, c.hid
    x = np.asarray(x, np.float32)
    ei = np.asarray(edge_index, np.int64)
    loops = np.arange(N, dtype=np.int64)
    src = np.concatenate([ei[0], loops])
    dst = np.concatenate([ei[1], loops])

    gmap = _gid_map(c)
    gsrc = _haddr(c, gmap[src])       # table row address of the source
    gdst = gmap[dst]
    core = gdst // c.pcpad
    loc = gdst % c.pcpad
    blk = loc // 128
    dstloc = loc % 128
    isB = (gsrc >= c.split).astype(np.int64)

    # dst-major within (core, blk, half) so the per-tile dst one-hot is a
    # staircase; addr-minor for residual gather locality.
    order = np.lexsort((gsrc, dstloc, isB, blk, core))
    gsrc, core, blk, dstloc, isB = (
        a[order] for a in (gsrc, core, blk, dstloc, isB))

    counts = np.zeros((C, c.bpc, 2), np.int64)
    np.add.at(counts, (core, blk, isB), 1)
    ntiles = -(-counts // 128)
    nA = ntiles[:, :, 0].max(axis=0)
    nB = ntiles[:, :, 1].max(axis=0)
    TA, TB = int(nA.sum()), int(nB.sum())
    T = TA + TB
    offA = np.concatenate([[0], np.cumsum(nA)]).astype(np.int64)
    offB = np.concatenate([[0], np.cumsum(nB)]).astype(np.int64)

    padidx = -1 if c.negpad else 0
    idxA = np.full((C, max(TA, 1) * 128), padidx, np.int64)
    idxB = np.full((C, max(TB, 1) * 128), padidx, np.int64)
    dlA = np.full((C, max(TA, 1) * 128), -1.0, np.float32)
    dlB = np.full((C, max(TB, 1) * 128), -1.0, np.float32)
    padA = np.zeros((C, c.bpc), np.int64)   # real count in last tile (0=full)
    padB = np.zeros((C, c.bpc), np.int64)

    starts = np.concatenate([[0], np.cumsum(counts.reshape(-1))])[:-1]
    starts = starts.reshape(C, c.bpc, 2)
    for ci in range(C):
        for b in range(c.bpc):
            n0 = counts[ci, b, 0]
            s0 = starts[ci, b, 0]
            a0 = offA[b] * 128
            idxA[ci, a0:a0 + n0] = gsrc[s0:s0 + n0]
            dlA[ci, a0:a0 + n0] = dstloc[s0:s0 + n0]
            padA[ci, b] = n0 % 128 if n0 % 128 else (128 if nA[b] else 0)
            n1 = counts[ci, b, 1]
            s1 = starts[ci, b, 1]
            b0_ = offB[b] * 128
            idxB[ci, b0_:b0_ + n1] = gsrc[s1:s1 + n1] - c.split
            dlB[ci, b0_:b0_ + n1] = dstloc[s1:s1 + n1]
            padB[ci, b] = n1 % 128 if n1 % 128 else (128 if nB[b] else 0)

    # consumption-order tables: per block, A tiles then B tiles.
    # dst_sl is the per-slot dst replicated across partitions: the one-hot
    # S2[j,t,p] = (dst_sl[j, t*128+p] == j) is a single packed-2B is_equal.
    dst_cols = np.zeros((C, 128, max(T, 1)), np.float32)
    dst_sl = np.zeros((C, max(T, 1) * 128), np.float32)
    tpos = 0
    for b in range(c.bpc):
        na, nb = int(nA[b]), int(nB[b])
        a0, b0_ = offA[b] * 128, offB[b] * 128
        for ci in range(C):
            seg = np.concatenate(
                [dlA[ci, a0:a0 + na * 128], dlB[ci, b0_:b0_ + nb * 128]])
            segm = seg.reshape(na + nb, 128)
            dst_cols[ci, :, tpos:tpos + na + nb] = segm.T
            dst_sl[ci, tpos * 128:(tpos + na + nb) * 128] = seg
        tpos += na + nb

    # own-address index lists for the adloc build (lo/hi halves; pads -> the
    # reserved zero row of the other half so gsum = glo + ghi works)
    own = np.zeros((C, c.pcpad), np.int64)
    for ci in range(C):
        own[ci] = _haddr(c, ci * c.pcpad + np.arange(c.pcpad))
    own_lo = np.where(own < c.split, own, 0)
    own_hi = np.where(own >= c.split, own - c.split, 0)

    import ml_dtypes
    bf = ml_dtypes.bfloat16
    per_core = []
    for ci in range(C):
        per_core.append(dict(
            idxA=_wrap_idx(idxA[ci]),
            idxB=_wrap_idx(idxB[ci]),
            idxOlo=_wrap_idx(own_lo[ci]),
            idxOhi=_wrap_idx(own_hi[ci]),
            dst_cols=np.ascontiguousarray(dst_cols[ci]).astype(bf),
            dst_sl=np.broadcast_to(
                dst_sl[ci].astype(bf)[None, :],
                (128, dst_sl.shape[1])).copy(),
        ))

    xT = np.zeros((c.in_ch, c.npad), np.float32)
    xT[:, gmap] = x.T

    W0 = np.asarray(W0, np.float32)
    As = np.einsum("ihc,hc->ih", W0.reshape(c.in_ch, H, HID),
                   np.asarray(a_src0, np.float32))
    Ad = np.einsum("ihc,hc->ih", W0.reshape(c.in_ch, H, HID),
                   np.asarray(a_dst0, np.float32))
    # (c, h)-interleaved channel order: col' = c*H + h <- col = h*HID + c.
    # Keeps the per-head e broadcast packed in the last AP dim (DVE 2x).
    perm = (np.arange(c.val_w).reshape(H, HID).T.reshape(-1))
    W0p = W0[:, perm]
    wpack = np.concatenate([W0p, As, Ad], axis=1).astype(np.float32)

    W1 = np.asarray(W1, np.float32)
    ws = (W1 @ np.asarray(a_src1, np.float32)[0]).astype(np.float32)
    wd = (W1 @ np.asarray(a_dst1, np.float32)[0]).astype(np.float32)
    # projection matrix for the flush: z @ [ws | wd | W1c0 | W1c1],
    # rows in permuted channel order, split into two 128-row chunks
    wproj = np.stack([ws, wd, W1[:, 0], W1[:, 1]], axis=1)[perm]
    wproj = np.concatenate([wproj[0:128], wproj[128:256]], axis=1)  # [128,8]

    def rep(v, p=True):
        v = np.asarray(v, np.float32)
        if p:
            v = v[perm]
        return np.broadcast_to(v[None, :], (128, v.shape[0])).copy()

    shared = dict(
        xT=xT.astype(bf), wpack=wpack.astype(bf),
        b0rep=rep(b0), grep=rep(ln_g), brep=rep(ln_b),
        wsrep=rep(ws), wdrep=rep(wd),
        b1rep=rep(np.asarray(b1, np.float32), p=False),
        w1c0rep=rep(W1[:, 0]), w1c1rep=rep(W1[:, 1]),
        iota=np.broadcast_to(
            np.arange(128, dtype=np.float32)[None, :], (128, 128))
        .astype(bf).copy(),
        iotaP=np.broadcast_to(
            np.arange(128, dtype=np.float32)[:, None], (128, 128))
        .astype(bf).copy(),
        wproj=wproj.astype(bf),
        ident=np.eye(128, dtype=np.float32),
    )
    sched = dict(nA=nA, nB=nB, offA=offA, offB=offB, TA=TA, TB=TB, T=T,
                 padA=padA.min(axis=0), padB=padB.min(axis=0))
    return sched, shared, per_core, gmap


# ----------------------------------------------------------------- build ----

def build(sched, stage="full", sub=99, repeat=1):
    """stage: one of ph0, adloc, l0, ag, exp, full — truncate for bisect.
    sub: 1=gathers only, 2=+e chain, 4=+matmuls, 99=full."""
    global _SUB
    _SUB = sub
    c = cfg
    H = c.heads
    VW = c.val_w
    VR = VW + 128            # layer-0 row elems (bf16): 768B
    AC = VW                  # as/ad fp32 bit-packed at [AC, AC+4H)
    L1W = 128                # layer-1 row elems (bf16): 256B
    nA, nB = sched["nA"], sched["nB"]
    offA, offB = sched["offA"], sched["offB"]
    TA, TB, T = sched["TA"], sched["TB"], sched["T"]

    nc = bacc.Bacc("TRN2", target_bir_lowering=False, debug=False,
                   num_devices=c.n_cores,
                   num_swdge_queues=4, dynamic_dma_scratch_size=65536)

    def inp(name, shape, dt):
        return nc.dram_tensor(name, list(shape), dt, kind="ExternalInput")

    xT_d = inp("xT", (c.in_ch, c.npad), BF16)
    wpack_d = inp("wpack", (c.in_ch, VW + 2 * H), BF16)
    w1c0_d = inp("w1c0rep", (128, VW), F32)
    w1c1_d = inp("w1c1rep", (128, VW), F32)
    b0_d = inp("b0rep", (128, VW), F32)
    g_d = inp("grep", (128, VW), F32)
    bln_d = inp("brep", (128, VW), F32)
    ws_d = inp("wsrep", (128, VW), F32)
    wd_d = inp("wdrep", (128, VW), F32)
    b1_d = inp("b1rep", (128, c.out_ch), F32)
    iota_d = inp("iota", (128, 128), BF16)
    idxA_d = inp("idxA", (128, max(TA, 1) * 8), I16)
    idxB_d = inp("idxB", (128, max(TB, 1) * 8), I16)
    idxOlo_d = inp("idxOlo", (128, c.pcpad // 16), I16)
    idxOhi_d = inp("idxOhi", (128, c.pcpad // 16), I16)
    dst_d = inp("dst_cols", (128, max(T, 1)), BF16)
    dsl_d = inp("dst_sl", (128, max(T, 1) * 128), BF16)
    iotaP_d = inp("iotaP", (128, 128), BF16)
    wproj_d = inp("wproj", (128, 8), BF16)
    ident_d = inp("ident", (128, 128), F32)

    val0 = nc.dram_tensor("val0", [c.npad, VR], BF16)
    val1 = nc.dram_tensor("val1", [c.npad, L1W], BF16)
    v1loc = nc.dram_tensor("v1loc", [c.pcpad, 4], BF16)
    v1glob = nc.dram_tensor("v1glob", [c.npad, 4], BF16, addr_space="Shared")
    outp = nc.dram_tensor("outp", [c.pcpad, c.out_ch], F32,
                          kind="ExternalOutput")
    dbg = nc.dram_tensor("dbg", [128, 4096], F32)

    groups = list(range(0, c.bpc, c.gb))
    _order = ["ph0", "adloc", "l0", "ag", "exp", "full"]
    _lvl = _order.index(stage)

    with tile.TileContext(nc) as tc:
        with (
            tc.tile_pool(name="const", bufs=1) as cpool,
            tc.tile_pool(name="gath", bufs=c.gbufs) as gpool,
            tc.tile_pool(name="idx", bufs=c.gbufs) as ipool,
            tc.tile_pool(name="s2p", bufs=2) as s2pool,
            tc.tile_pool(name="psad", bufs=1, space="PSUM") as psad,
            tc.tile_pool(name="pspj", bufs=1, space="PSUM") as pspj,
            tc.tile_pool(name="work", bufs=2) as wpool,
            tc.tile_pool(name="psagg", bufs=c.aggbufs, space="PSUM") as psagg,
            tc.tile_pool(name="flush", bufs=c.fbufs) as fpool,
        ):
            def cload(ap, shape, dt, tag):
                t = cpool.tile(shape, dt, tag=tag)
                nc.sync.dma_start(t[:], ap[:])
                return t

            wpack_t = cload(wpack_d, [c.in_ch, VW + 2 * H], BF16, "wpack")
            b0_t = cload(b0_d, [128, VW], F32, "b0")
            g_t = cload(g_d, [128, VW], F32, "g")
            bln_t = cload(bln_d, [128, VW], F32, "bln")
            ws_t = cload(ws_d, [128, VW], F32, "ws")
            wd_t = cload(wd_d, [128, VW], F32, "wd")
            b1_t = cload(b1_d, [128, c.out_ch], F32, "b1")
            dst_t = cload(dst_d, [128, max(T, 1)], BF16, "dstc")
            iota_t = cload(iota_d, [128, 128], BF16, "iota")
            iotaP_t = cload(iotaP_d, [128, 128], BF16, "iotaP")
            wproj_t = cload(wproj_d, [128, 8], BF16, "wproj")
            ident_t = cload(ident_d, [128, 128], F32, "ident")
            w1c0_t = cload(w1c0_d, [128, VW], F32, "w1c0")
            w1c1_t = cload(w1c1_d, [128, VW], F32, "w1c1")
            # persistent alpha-dst tables (SBUF-resident)
            gsum = cpool.tile([128, c.bpc, 2 * H], F32, tag="gsum")
            adbf = cpool.tile([128, c.bpc, H], BF16, tag="adbf")
            a1bf = cpool.tile([128, c.bpc, 1], BF16, tag="a1bf")
            v1sb = cpool.tile([128, c.bpc, 4], BF16, tag="v1sb")

            def emit_body():
                # ---------------- phase 0: node transform for all gids ------
                nblk_tot = c.n_cores * c.bpc
                CH0 = c.ch0
                with (
                    tc.tile_pool(name="xchunk", bufs=2) as xpool,
                    tc.tile_pool(name="ph0", bufs=2) as p0pool,
                    tc.tile_pool(name="psmm", bufs=2, space="PSUM") as psmm,
                ):
                    for bg in range(0, nblk_tot, CH0):
                        nbk = min(CH0, nblk_tot - bg)
                        xc = xpool.tile([c.in_ch, CH0 * 128], BF16, tag="xc")
                        nc.sync.dma_start(
                            xc[:, :nbk * 128],
                            xT_d[:, bg * 128:(bg + nbk) * 128])
                        hz = p0pool.tile([128, CH0, VR], BF16, tag="hz")
                        if bg < 2 * CH0:
                            nc.vector.memset(hz[:], 0.0)
                        if _SUB == 11:      # loads only
                            cv0 = p0pool.tile([128, 8], BF16, tag="cv0")
                            nc.vector.tensor_copy(cv0[:], xc[:, 0:8])
                            nc.sync.dma_start(
                                dbg[:, 0:4], cv0[:].bitcast(F32))
                            continue
                        for k in range(nbk):
                            ps = psmm.tile([128, VW + 2 * H], F32, tag="ph0ps")
                            nc.tensor.matmul(
                                ps[:], xc[:, k * 128:(k + 1) * 128],
                                wpack_t[:], start=True, stop=True)
                            if _SUB == 12:  # +matmuls, consume psum
                                nc.vector.tensor_copy(
                                    hz[:, k, 0:8].bitcast(F32), ps[:, 0:4])
                                continue
                            nc.scalar.activation(hz[:, k, 0:VW], ps[:, 0:VW],
                                                 ACTF.Copy)
                            nc.vector.tensor_copy(
                                hz[:, k, AC:AC + 4 * H].bitcast(F32),
                                ps[:, VW:VW + 2 * H])
                        if _SUB in (12, 13):  # no store
                            continue
                        # p-major address layout: row (p*nblk + k) -> the
                        # write is contiguous per partition.  Full 768B rows
                        # (cols 272:384 are never read) keep it unstrided.
                        dst_view = val0.rearrange("(p k) w -> p k w", p=128)[
                            :, bg:bg + nbk, :]
                        nc.sync.dma_start(dst_view, hz[:, :nbk, :])

                # ---------------- adloc: own rows' as/ad via lo+hi gathers --
                ntile_own = c.pcpad // 128
                if _lvl >= 1:
                    with tc.tile_pool(name="adbuild", bufs=2) as apool:
                        olo_t = apool.tile([128, c.pcpad // 16], I16,
                                           tag="olo")
                        nc.sync.dma_start(olo_t[:], idxOlo_d[:])
                        ohi_t = apool.tile([128, c.pcpad // 16], I16,
                                           tag="ohi")
                        nc.sync.dma_start(ohi_t[:], idxOhi_d[:])
                        CH = 5  # tiles per chunk
                        for t0 in range(0, ntile_own, CH):
                            t1 = min(t0 + CH, ntile_own)
                            nt = t1 - t0
                            glo = apool.tile([128, CH, VR], BF16, tag="glo")
                            nc.gpsimd.dma_gather(
                                out_ap=glo[:, :nt, :],
                                in_ap=val0[0:c.split, :],
                                idxs_ap=olo_t[:, t0 * 8:t1 * 8],
                                num_idxs=nt * 128,
                                num_idxs_reg=nt * 128, elem_size=VR,
                                single_packet=False, queue_num=0)
                            ghi = apool.tile([128, CH, VR], BF16, tag="ghi")
                            nc.gpsimd.dma_gather(
                                out_ap=ghi[:, :nt, :],
                                in_ap=val0[c.split:c.npad, :],
                                idxs_ap=ohi_t[:, t0 * 8:t1 * 8],
                                num_idxs=nt * 128,
                                num_idxs_reg=nt * 128, elem_size=VR,
                                single_packet=False, queue_num=1)
                            nc.vector.tensor_tensor(
                                gsum[:, t0:t1, :],
                                glo[:, :nt, AC:AC + 4 * H].bitcast(F32),
                                ghi[:, :nt, AC:AC + 4 * H].bitcast(F32),
                                ALU.add)
                        nc.vector.tensor_copy(adbf[:], gsum[:, :, H:2 * H])

                # ---------------- one attention layer -----------------------
                def emit_layer(lyr):
                    HL = H if lyr == 0 else 1
                    valt = val0 if lyr == 0 else val1
                    VRl = VR if lyr == 0 else L1W
                    NCOL = VW + H if lyr == 0 else 3
                    EC = VW if lyr == 0 else 2
                    adtab = adbf if lyr == 0 else a1bf
                    for g0 in groups:
                        g1 = min(g0 + c.gb, c.bpc)
                        blks = range(g0, g1)
                        tA0, tA1 = int(offA[g0]), int(offA[g1])
                        tB0, tB1 = int(offB[g0]), int(offB[g1])
                        nAg, nBg = tA1 - tA0, tB1 - tB0
                        nG = nAg + nBg
                        if nG == 0:
                            continue
                        tD0 = tA0 + tB0
                        gq = (g0 // c.gb) % 2
                        if nAg:
                            iA = ipool.tile([128, nAg * 8], I16, tag="iA")
                            nc.sync.dma_start(iA[:],
                                              idxA_d[:, tA0 * 8:tA1 * 8])
                        if nBg:
                            iB = ipool.tile([128, nBg * 8], I16, tag="iB")
                            nc.sync.dma_start(iB[:],
                                              idxB_d[:, tB0 * 8:tB1 * 8])
                        dsl = ipool.tile([128, nG * 128], BF16, tag="dsl")
                        nc.sync.dma_start(
                            dsl[:], dsl_d[:, tD0 * 128:(tD0 + nG) * 128])

                        vA = gpool.tile([128, max(nAg, 1), VRl], BF16,
                                        tag="vA")
                        vB = gpool.tile([128, max(nBg, 1), VRl], BF16,
                                        tag="vB")
                        if nAg:
                            nc.gpsimd.dma_gather(
                                out_ap=vA[:], in_ap=valt[0:c.split, :],
                                idxs_ap=iA[:], num_idxs=nAg * 128,
                                num_idxs_reg=nAg * 128, elem_size=VRl,
                                single_packet=False, queue_num=gq)
                        if nBg:
                            nc.gpsimd.dma_gather(
                                out_ap=vB[:], in_ap=valt[c.split:c.npad, :],
                                idxs_ap=iB[:], num_idxs=nBg * 128,
                                num_idxs_reg=nBg * 128, elem_size=VRl,
                                single_packet=False, queue_num=1 - gq)

                        if _SUB == 1:   # consume gathers, skip compute
                            cv = wpool.tile([128, 64], F32, tag="cv")
                            nc.vector.tensor_copy(
                                cv[:], vA[:, 0, 0:128].bitcast(F32))
                            nc.sync.dma_start(dbg[:, 0:64], cv[:])
                            if nBg:
                                nc.vector.tensor_copy(
                                    cv[:], vB[:, 0, 0:128].bitcast(F32))
                                nc.sync.dma_start(dbg[:, 64:128], cv[:])
                            continue

                        # S2[j, t, p] = (dst_sl[j, t*128+p] == j): single
                        # is_equal with both operands packed 2B (DVE 2x)
                        s2 = s2pool.tile([128, nG, 128], BF16, tag="s2")
                        nc.vector.tensor_tensor(
                            s2[:],
                            dsl[:].rearrange("p (t j) -> p t j", j=128),
                            iotaP_t[:].unsqueeze(1)
                            .broadcast_to([128, nG, 128]),
                            ALU.is_equal)
                        # adE[p, t, h] = sum_j S2[j,t,p] * adtab[j, blk, h]
                        adE = psad.tile([128, nG, HL], F32, tag="adE")
                        pos = 0
                        for b in blks:
                            for _ in range(int(nA[b]) + int(nB[b])):
                                nc.tensor.matmul(
                                    adE[:, pos, :], s2[:, pos, :],
                                    adtab[:, b, 0:HL], start=True, stop=True)
                                pos += 1

                        # logit = as(gathered) + adE, per (block, half) seg
                        logit = wpool.tile([128, nG, HL], F32, tag="logit")
                        pos = 0
                        for b in blks:
                            na, nb_ = int(nA[b]), int(nB[b])
                            if na:
                                sA = int(offA[b]) - tA0
                                if lyr == 0:
                                    asv = vA[:, sA:sA + na, AC:AC + 4 * H]\
                                        .bitcast(F32)[:, :, 0:HL]
                                else:
                                    asv = vA[:, sA:sA + na, 2:4].bitcast(F32)
                                nc.vector.tensor_tensor(
                                    logit[:, pos:pos + na, :], asv,
                                    adE[:, pos:pos + na, :], ALU.add)
                                pos += na
                            if nb_:
                                sB = int(offB[b]) - tB0
                                if lyr == 0:
                                    bsv = vB[:, sB:sB + nb_, AC:AC + 4 * H]\
                                        .bitcast(F32)[:, :, 0:HL]
                                else:
                                    bsv = vB[:, sB:sB + nb_, 2:4]\
                                        .bitcast(F32)
                                nc.vector.tensor_tensor(
                                    logit[:, pos:pos + nb_, :], bsv,
                                    adE[:, pos:pos + nb_, :], ALU.add)
                                pos += nb_
                        lr = wpool.tile([128, nG, HL], F32, tag="lr")
                        nc.vector.scalar_tensor_tensor(
                            lr[:], logit[:], float(c.neg), logit[:],
                            ALU.mult, ALU.max)
                        if lyr == 1:
                            w4 = wpool.tile([128, nG, 4], BF16, tag="w4")
                            nc.scalar.activation(
                                w4[:, :, 2:3], lr[:], ACTF.Exp)
                        pos = 0
                        for b in blks:
                            na, nb_ = int(nA[b]), int(nB[b])
                            if na:
                                sA = int(offA[b]) - tA0
                                if lyr == 0:
                                    nc.scalar.activation(
                                        vA[:, sA:sA + na, EC:EC + HL],
                                        lr[:, pos:pos + na, :], ACTF.Exp)
                                pos += na
                            if nb_:
                                sB = int(offB[b]) - tB0
                                if lyr == 0:
                                    nc.scalar.activation(
                                        vB[:, sB:sB + nb_, EC:EC + HL],
                                        lr[:, pos:pos + nb_, :], ACTF.Exp)
                                pos += nb_

                        if _SUB == 2:   # stop after e chain
                            ce = wpool.tile([128, nG], F32, tag="ce")
                            nc.vector.tensor_copy(ce[:], lr[:, :, 0])
                            nc.sync.dma_start(dbg[:, 0:nG], ce[:])
                            continue

                        # V' = V * e (per head), in A/B stream order
                        pos = 0
                        for b in blks:
                            na, nb_ = int(nA[b]), int(nB[b])
                            if na:
                                sA = int(offA[b]) - tA0
                                if lyr == 0:
                                    # (c, h)-interleaved values: per-head e
                                    # broadcast keeps the last dim packed
                                    vv = vA[:, sA:sA + na, 0:VW].rearrange(
                                        "p t (ch h) -> p t ch h", h=HL)
                                    ee = vA[:, sA:sA + na, EC:EC + HL]\
                                        .unsqueeze(2)
                                    nc.vector.tensor_tensor(
                                        vv, vv, ee.broadcast_to(
                                            [128, na, VW // HL, HL]),
                                        ALU.mult)
                                else:
                                    nc.vector.tensor_tensor(
                                        w4[:, pos:pos + na, 0:2],
                                        vA[:, sA:sA + na, 0:2],
                                        w4[:, pos:pos + na, 2:3].broadcast_to(
                                            [128, na, 2]),
                                        ALU.mult)
                                pos += na
                            if nb_:
                                sB = int(offB[b]) - tB0
                                if lyr == 0:
                                    vv = vB[:, sB:sB + nb_, 0:VW].rearrange(
                                        "p t (ch h) -> p t ch h", h=HL)
                                    ee = vB[:, sB:sB + nb_, EC:EC + HL]\
                                        .unsqueeze(2)
                                    nc.vector.tensor_tensor(
                                        vv, vv, ee.broadcast_to(
                                            [128, nb_, VW // HL, HL]),
                                        ALU.mult)
                                else:
                                    nc.vector.tensor_tensor(
                                        w4[:, pos:pos + nb_, 0:2],
                                        vB[:, sB:sB + nb_, 0:2],
                                        w4[:, pos:pos + nb_, 2:3]
                                        .broadcast_to([128, nb_, 2]),
                                        ALU.mult)
                                pos += nb_

                        # per block: S build, matmul accumulate, flush
                        pos = 0
                        for b in blks:
                            na, nb_ = int(nA[b]), int(nB[b])
                            tb = na + nb_
                            if tb == 0:
                                continue
                            tcol0 = tD0 + pos
                            s_t = wpool.tile([128, tb * 128], BF16, tag="S")
                            nc.vector.tensor_tensor(
                                s_t[:].rearrange("p (t j) -> p t j", j=128),
                                dst_t[:, tcol0:tcol0 + tb].unsqueeze(2)
                                .broadcast_to([128, tb, 128]),
                                iota_t[:].unsqueeze(1)
                                .broadcast_to([128, tb, 128]),
                                ALU.is_equal)
                            agg = psagg.tile([128, NCOL], F32, tag="agg")
                            for t in range(tb):
                                lhs = s_t[:, t * 128:(t + 1) * 128]
                                if lyr == 1:
                                    vv = w4[:, pos + t, 0:NCOL]
                                elif t < na:
                                    vv = vA[:, int(offA[b]) - tA0 + t,
                                            0:NCOL]
                                else:
                                    vv = vB[:, int(offB[b]) - tB0 + (t - na),
                                            0:NCOL]
                                st_, sp_ = (t == 0), (t == tb - 1)
                                nc.tensor.matmul(agg[:], lhs, vv,
                                                 start=st_, stop=sp_)
                            pos += tb

                            if _SUB == 4:   # stop after matmuls
                                ca = fpool.tile([128, NCOL], F32, tag="ca")
                                nc.vector.tensor_copy(ca[:], agg[:])
                                nc.sync.dma_start(dbg[:, 0:NCOL], ca[:])
                                continue

                            # ---- flush this block
                            den = agg[:, EC:EC + HL]
                            deneps = fpool.tile([128, HL], F32, tag="deneps")
                            nc.vector.tensor_scalar_add(deneps[:], den,
                                                        1e-30)
                            rcp = fpool.tile([128, HL], F32, tag="rcp")
                            nc.vector.reciprocal(rcp[:], deneps[:])
                            VWl = VW if lyr == 0 else 2
                            sc = fpool.tile([128, VWl], F32, tag="sc")
                            if lyr == 0:
                                nc.vector.tensor_tensor(
                                    sc[:].rearrange(
                                        "p (ch h) -> p ch h", h=HL),
                                    agg[:, 0:VWl].rearrange(
                                        "p (ch h) -> p ch h", h=HL),
                                    rcp[:].unsqueeze(1).broadcast_to(
                                        [128, VWl // HL, HL]),
                                    ALU.mult)
                            else:
                                nc.vector.tensor_tensor(
                                    sc[:], agg[:, 0:VWl],
                                    rcp[:].broadcast_to([128, VWl]),
                                    ALU.mult)
                            if lyr == 0:
                                flush0(sc, b)
                            else:
                                flush1(sc, b)

                # ---- layer-0 flush: +b0, LayerNorm, ELU, projections -------
                def flush0(sc, b):
                    nc.vector.tensor_tensor(sc[:], sc[:], b0_t[:], ALU.add)
                    mu = fpool.tile([128, 1], F32, tag="mu")
                    nc.vector.tensor_reduce(
                        mu[:], sc[:], mybir.AxisListType.X, ALU.add)
                    xc_ = fpool.tile([128, VW], F32, tag="xc0")
                    nc.vector.scalar_tensor_tensor(
                        xc_[:], mu[:].broadcast_to([128, VW]), -1.0 / VW,
                        sc[:], ALU.mult, ALU.add)
                    sq = fpool.tile([128, VW], F32, tag="sq")
                    nc.vector.tensor_tensor(sq[:], xc_[:], xc_[:], ALU.mult)
                    var = fpool.tile([128, 1], F32, tag="var")
                    nc.vector.tensor_reduce(
                        var[:], sq[:], mybir.AxisListType.X, ALU.add)
                    nc.vector.tensor_scalar(
                        var[:], var[:], 1.0 / VW, float(c.eps),
                        ALU.mult, ALU.add)
                    sd = fpool.tile([128, 1], F32, tag="sd")
                    nc.scalar.activation(sd[:], var[:], ACTF.Sqrt)
                    rstd = fpool.tile([128, 1], F32, tag="rstd")
                    nc.vector.reciprocal(rstd[:], sd[:])
                    zz = fpool.tile([128, VW], F32, tag="zz")
                    nc.vector.scalar_tensor_tensor(
                        zz[:], xc_[:], rstd[:], g_t[:], ALU.mult, ALU.mult)
                    nc.vector.tensor_tensor(zz[:], zz[:], bln_t[:], ALU.add)
                    # ELU: z = max(x,0) + exp(min(x,0)) - 1
                    zmin = fpool.tile([128, VW], F32, tag="zmin")
                    nc.vector.tensor_scalar_min(zmin[:], zz[:], 0.0)
                    pexp = fpool.tile([128, VW], F32, tag="pexp")
                    nc.scalar.activation(pexp[:], zmin[:], ACTF.Exp)
                    zmax = fpool.tile([128, VW], F32, tag="zmax")
                    nc.vector.tensor_scalar_max(zmax[:], zz[:], 0.0)
                    z = fpool.tile([128, VW], F32, tag="z")
                    nc.vector.tensor_tensor(z[:], zmax[:], pexp[:], ALU.add)
                    nc.vector.tensor_scalar_add(z[:], z[:], -1.0)
                    # attention scalars + layer-1 channels on PE:
                    # proj = z @ [ws|wd|W1c0|W1c1] via transpose + 2 matmuls
                    zTsb = fpool.tile([128, 2, 128], BF16, tag="zTsb")
                    proj = pspj.tile([128, 4], F32, tag="proj")
                    for k in range(2):
                        zT = pspj.tile([128, 128], F32, tag="zT")
                        nc.tensor.transpose(
                            zT[:], z[:, k * 128:(k + 1) * 128], ident_t[:])
                        nc.scalar.activation(zTsb[:, k, :], zT[:], ACTF.Copy)
                    for k in range(2):
                        nc.tensor.matmul(
                            proj[:], zTsb[:, k, :],
                            wproj_t[:, k * 4:(k + 1) * 4],
                            start=(k == 0), stop=(k == 1))
                    nc.vector.tensor_copy(v1sb[:, b, 0:2], proj[:, 2:4])
                    nc.vector.tensor_copy(
                        v1sb[:, b, 2:4].bitcast(F32), proj[:, 0:1])
                    nc.vector.tensor_copy(a1bf[:, b, 0:1], proj[:, 1:2])

                # ---- layer-1 flush: +b1, store -----------------------------
                def flush1(sc, b):
                    rows = slice(b * 128, (b + 1) * 128)
                    ob = fpool.tile([128, c.out_ch], F32, tag="ob")
                    nc.vector.tensor_tensor(
                        ob[:], sc[:, 0:c.out_ch], b1_t[:], ALU.add)
                    nc.sync.dma_start(outp[rows, :], ob[:])

                if _lvl >= 2:
                    emit_layer(0)
                    # single contiguous write of the compact local table
                    # (local addr = p * bpc + blk, matching v1sb layout)
                    if _SUB == 99:
                        nc.sync.dma_start(
                            v1loc.rearrange("(p k) w -> p k w", p=128),
                            v1sb[:])

                if _lvl >= 3:
                    nc.gpsimd.collective_compute(
                        "AllGather", ALU.bypass,
                        replica_groups=[list(range(c.n_cores))],
                        ins=[v1loc[:]], outs=[v1glob[:]])

                if _lvl >= 4:
                    # expand compact v1glob into the 256B-stride gather
                    # table.  v1glob row ci*pcpad + p*bpc + blk maps to val1
                    # row p*nblk_tot + ci*bpc + blk: both sides contiguous
                    # per partition when chunked by source core.
                    val1_v = val1.rearrange("(p kk) w -> p kk w", p=128)
                    EX = (c.bpc + 1) // 2
                    with (
                        tc.tile_pool(name="exs", bufs=2) as exs,
                        tc.tile_pool(name="exb", bufs=2) as exb,
                    ):
                        for cx in range(2 * c.n_cores):
                            ci, sub = cx // 2, cx % 2
                            k0 = sub * EX
                            nbk = min(EX, c.bpc - k0)
                            if nbk <= 0:
                                continue
                            sm = exs.tile([128, EX, 4], BF16, tag="sm")
                            src = v1glob[ci * c.pcpad:(ci + 1) * c.pcpad, :]\
                                .rearrange("(p k) w -> p k w", p=128)
                            nc.sync.dma_start(sm[:, :nbk, :],
                                              src[:, k0:k0 + nbk, :])
                            big = exb.tile([128, EX, L1W], BF16, tag="big")
                            if cx < 2:
                                nc.vector.memset(big[:], 0.0)
                            nc.vector.tensor_copy(big[:, :nbk, 0:4],
                                                  sm[:, :nbk, :])
                            nc.sync.dma_start(
                                val1_v[:, ci * c.bpc + k0:
                                       ci * c.bpc + k0 + nbk, :],
                                big[:, :nbk, :])

                if _lvl >= 5:
                    emit_layer(1)

            for _rep in range(repeat):
                emit_body()

    nc.compile()
    return nc


# ------------------------------------------------------------------ run -----

_cache = {}
_SUB = 99


def kernel(**inputs):
    c = cfg
    sched, shared, per_core, gmap = prep(**inputs)
    key = _sched_sig(sched)
    if key in _cache:
        nc = _cache[key]
    else:
        nc = build(sched)
        _cache[key] = nc

    in_maps = []
    for ci in range(c.n_cores):
        m = dict(shared)
        m.update(per_core[ci])
        in_maps.append(m)
    res = run_bass_kernel_spmd(nc, in_maps, list(range(c.n_cores)))
    out = np.zeros((c.n_nodes, c.out_ch), np.float32)
    loc = gmap % c.pcpad
    for ci in range(c.n_cores):
        sel = slice(ci * c.pc, (ci + 1) * c.pc)
        out[sel] = res.results[ci]["outp"][loc[sel]]
    return out.astype(np.float32)


def _sched_sig(s):
    return (tuple(s["nA"].tolist()), tuple(s["nB"].tolist()))
